# revision 45
# baseline (speedup 1.0000x reference)
"""EnhancedLoRALinear Trainium2 kernel (v2: bf16 main + fp8 DoubleRow gate/down).

Computes, for x:[4,8192,1024] and torch-style weights (out,in):
    out = x @ (W + W_res)^T + b + sigmoid(x @ W_gate^T) * (2 * (x @ W_down^T) @ W_up^T)

Strategy:
  - Data-parallel: the 32768 tokens are split across 8 NeuronCores (4096 each);
    the small weight matrices are replicated.
  - Algebraic fold: main + residual share one matmul with Wc = W + W_res.
  - Precision split (rel-err budget is 2e-2; measured 3.8e-3 on host):
      * main path in bf16 (x bf16 stationary, Wc bf16 moving) - full PE rate
        with FWL weight loads.
      * gate path in fp8 e4m3 with perf_mode=DoubleRow: K=1024 contraction in
        4 matmuls of K=256 (2 k-elements per cell). W_gate is scaled x64 so
        its entries leave the fp8 subnormal range; the sigmoid applies
        scale=1/64 to undo it. Sigmoid squashes the residual quantization
        error and the gate only multiplies the small LoRA term.
      * down-projection in fp8 DoubleRow too (W_down scaled x64; the 1/64
        plus the LoRA scaling 2.0 are folded into W_up host-side).
      * lora-up in bf16, with the two 512-wide output halves packed into
        concurrent row-tiled matmuls (K=16 each, rows 0-15 and 32-47).
  - DMA: weights issue on the Sync queue while x tiles issue on the Scalar
    queue (each dma_start costs ~1us of issue time on its engine; the v1
    kernel serialized everything on Sync which delayed the first real matmul
    to ~30us). Output tiles go back on Sync.
  - A short junk-matmul spin keeps the PE busy through the DMA prologue so
    the HAM clock gate is open when real work starts; ordering-only deps pin
    every junk matmul before the first matmul of each PSUM group.
"""

import ml_dtypes
import numpy as np

_BF16 = ml_dtypes.bfloat16
_F8E4 = ml_dtypes.float8_e4m3  # IEEE e4m3 (bias 7, max 240) == TRN FP8_EXP4

import concourse.bass as bass
import concourse.bacc as bacc
import concourse.mybir as mybir
import concourse.tile as tile
from concourse.bass_utils import run_bass_kernel_spmd
from concourse.tile_rust import add_dep_helper

N_CORES = 8
B, S = 4, 8192
TOK = B * S                  # 32768 tokens total
T = TOK // N_CORES           # 4096 tokens per core
I = 1024                     # in_features
O = 1024                     # out_features
R = 16                       # lora rank
SCALING = 2.0                # lora_alpha / r
KT = I // 128                # 8 bf16 contraction tiles
KJ = I // 256                # 4 fp8 DoubleRow contraction tiles
TG = 512                     # token group (down-projection batch)
NG = T // TG                 # 8 groups per core
NH = O // 512                # 2 output halves
WS = 64.0                    # fp8 weight scale (exact power of two)
N_JUNK = 12                  # HAM warm-up matmuls (cover the DMA prologue)

F32 = mybir.dt.float32
BF16 = mybir.dt.bfloat16
F8 = mybir.dt.float8e4
DR = mybir.MatmulPerfMode.DoubleRow


def _build_nc():
    nc = bacc.Bacc(None)

    # Pre-swizzled DRAM layouts, group-/half-major so every DMA moves 128
    # contiguous 4-8KB rows (small descriptors throttle the DMA queues):
    #   bf16 x:   [g, 128p, 8k, TG]     k = kt*128 + p
    #   fp8  x:   [g, 128p, 4j, 2i, TG] k = j*256 + i*128 + p
    #   wc/wg:    [half, 128p, ..., 512]
    xtb = nc.dram_tensor("xtb", [NG, 128, KT, TG], BF16, kind="ExternalInput")
    x8 = nc.dram_tensor("x8", [NG, 128, KJ, 2, TG], F8, kind="ExternalInput")
    wct = nc.dram_tensor("wct", [NH, 128, KT, 512], BF16, kind="ExternalInput")
    wg8 = nc.dram_tensor("wg8", [NH, 128, KJ, 2, 512], F8,
                         kind="ExternalInput")
    wd8 = nc.dram_tensor("wd8", [128, KJ, 2, 128], F8, kind="ExternalInput")
    wut2 = nc.dram_tensor("wut2", [R, O], BF16, kind="ExternalInput")
    biasr = nc.dram_tensor("biasr", [1, O], F32, kind="ExternalInput")
    out = nc.dram_tensor("out", [T, O], F32, kind="ExternalOutput")

    sig = mybir.ActivationFunctionType.Sigmoid
    mult = mybir.AluOpType.mult
    add = mybir.AluOpType.add

    with tile.TileContext(nc) as tc:
        with (
            tc.tile_pool(name="wpool", bufs=1) as wpool,
            tc.tile_pool(name="xpool", bufs=3) as xpool,
            tc.tile_pool(name="opool", bufs=3) as opool,
            tc.tile_pool(name="epool", bufs=3) as epool,
            tc.tile_pool(name="psum", bufs=1, space="PSUM") as pp,
        ):
            # --- resident weights + group-0 x, split across BOTH DMA issue
            # queues (Sync + Scalar share ~400 GB/s; each alone gets ~200),
            # ordered by first use under the mains->loras->gates tile order.
            # wc/wg are separate per-half tiles so both DMA sides stay fully
            # contiguous (128 descriptors of 4-8KB each). ---
            wc_h = [wpool.tile([128, KT, 512], BF16, name=f"wc{h}")
                    for h in range(NH)]
            wg_h = [wpool.tile([128, KJ, 2, 512], F8, name=f"wg{h}")
                    for h in range(NH)]
            wd_sb = wpool.tile([128, KJ, 2, 128], F8)
            # lora operands are zero-padded to K=128 so the lora matmuls are
            # full-row (partial-row matmuls break LDWEIGHTS prefetching)
            wu_sb = wpool.tile([128, O], BF16)
            down_pers = wpool.tile([128, TG], BF16)
            bias_r = wpool.tile([1, O], F32)
            bias_bc = wpool.tile([128, O], F32)

            x0_8 = xpool.tile([128, KJ, 2, TG], F8, tag="x8", name="x8_0")
            x0_b = xpool.tile([128, KT, TG], BF16, tag="xtb", name="xtb0")
            nc.gpsimd.memset(wu_sb[:, :], 0.0)
            nc.gpsimd.memset(down_pers[:, :], 0.0)
            # Big transfers are split into partition halves: a single DMA
            # instruction leaves the queue's engines latency-bound (~110
            # GB/s); two in flight pipeline much better.
            def dma2(eng, dst, src):
                eng.dma_start(out=dst[0:64], in_=src[0:64])
                eng.dma_start(out=dst[64:128], in_=src[64:128])

            # sync queue: first wc half (feeds phase-1 immediately), then the
            # fp8 tensors needed from group 1 on
            dma2(nc.sync, wc_h[0], wct[0])
            dma2(nc.sync, wg_h[0], wg8[0])
            nc.sync.dma_start(out=wd_sb[:, :, :, :], in_=wd8[:, :, :, :])
            dma2(nc.sync, x0_8, x8[0])
            # scalar queue: bias first (the phase-1 drains need it), bf16 x
            # (the other phase-1 input), second wc half, lora weights,
            # second wg half
            nc.scalar.dma_start(out=bias_r[:, :], in_=biasr[:, :])
            dma2(nc.scalar, x0_b, xtb[0])
            dma2(nc.scalar, wc_h[1], wct[1])
            nc.scalar.dma_start(out=wu_sb[0:R, :], in_=wut2[:, :])
            dma2(nc.scalar, wg_h[1], wg8[1])
            nc.gpsimd.partition_broadcast(bias_bc[:, :], bias_r[0:1, :])

            # HAM spin-up: junk matmuls keep the PE busy through the DMA
            # prologue so the clock gate is open before real compute starts
            junk = wpool.tile([128, 512], BF16)
            nc.gpsimd.memset(junk[:, :], 0.0)
            warm = pp.tile([128, 512], F32, tag="warm")
            spin = None
            for _ in range(N_JUNK):
                spin = nc.tensor.matmul(warm[:, :], junk[:, 0:128], junk[:, :],
                                        start=True, stop=True)
            first_real = []  # first matmul of each psum tag's first group

            # x-tile DMAs issue on the Scalar queue, one group ahead of use
            x_tiles = {}

            def issue_x(g):
                x8_t = xpool.tile([128, KJ, 2, TG], F8, tag="x8",
                                  name=f"x8_{g}")
                nc.scalar.dma_start(
                    out=x8_t[:, :, :, :], in_=x8[g, :, :, :, :]
                )
                xt_t = xpool.tile([128, KT, TG], BF16, tag="xtb",
                                  name=f"xtb{g}")
                nc.scalar.dma_start(
                    out=xt_t[:, :, :], in_=xtb[g, :, :, :]
                )
                x_tiles[g] = (xt_t, x8_t)

            # --- phase 1: group-0 mains only. Only wc and bf16-x are needed,
            # so the PE gets real work while the rest of the 5.5MB prologue
            # is still in flight (the prologue is HBM-bandwidth-bound).
            # main+bias results are staged in SBUF; group 0's fp8 paths and
            # the final combine run as phase 2 after group 7. ---
            # rotate over six psum banks (the gate/lora tags are free during
            # phase 1) so the 8-matmul bursts pipeline instead of
            # serializing on the DVE drain of a single bank
            P1_BANKS = ["main0", "main1", "gate0", "gate1", "lora0", "lora1"]
            g0mb = {}
            ci = 0
            for oh in range(NH):
                osl = slice(oh * 512, (oh + 1) * 512)
                for t in range(TG // 128):
                    tsl = slice(t * 128, (t + 1) * 128)
                    mps = pp.tile([128, 512], F32, tag=P1_BANKS[ci % 6],
                                  name=f"p1ps{ci}")
                    for k in range(KT):
                        mm = nc.tensor.matmul(
                            mps[:, :],
                            x0_b[:, k, tsl],
                            wc_h[oh][:, k, :],
                            start=(k == 0),
                            stop=(k == KT - 1),
                        )
                        if ci < 6 and k == 0:
                            first_real.append(mm)
                    st_sb = wpool.tile([128, 512], F32, name=f"g0mb{t}_{oh}")
                    nc.vector.tensor_tensor(
                        st_sb[:, :], mps[:, :], bias_bc[:, osl], add
                    )
                    g0mb[(t, oh)] = st_sb
                    ci += 1

            issue_x(1)
            issue_x(2)
            for g in range(1, NG):
                tg0 = g * TG
                xt_sb, x8_sb = x_tiles.pop(g)

                # LoRA down-projection (64x scaled) for the whole group.
                # wd is zero-padded to 128 output columns so its LDWEIGHTS
                # keeps the full column mask (no prefetch break).
                dps = pp.tile([128, TG], F32, tag="dps")
                for j in range(KJ):
                    mm = nc.tensor.matmul(
                        dps[:, :],
                        wd_sb[:, j, :, :],
                        x8_sb[:, j, :, :],
                        start=(j == 0),
                        stop=(j == KJ - 1),
                        perf_mode=DR,
                    )
                    if g == 1 and j == 0:
                        first_real.append(mm)
                nc.vector.tensor_copy(down_pers[0:R, :], dps[0:R, :])

                for t in range(TG // 128):
                    # prefetch the next-next group's x mid-group, after the
                    # DGE queue has drained (issuing early blocks the scalar
                    # engine on descriptor backpressure, delaying sigmoids)
                    if t == 2 and g + 2 < NG:
                        issue_x(g + 2)
                    tsl = slice(t * 128, (t + 1) * 128)
                    out_sb = opool.tile([128, O], F32, tag="out")
                    # tile order: all bf16 work first (mains, loras), then
                    # the fp8 gates contiguously -- the fp8-DR weight path
                    # entry costs ~190ns, so pay it once per tile, adjacent
                    # to the next group's fp8 down-projection
                    mset = {}
                    for oh in range(NH):
                        osl = slice(oh * 512, (oh + 1) * 512)
                        mps = pp.tile([128, 512], F32, tag=f"main{oh}")
                        for k in range(KT):
                            mm = nc.tensor.matmul(
                                mps[:, :],
                                xt_sb[:, k, tsl],
                                wc_h[oh][:, k, :],
                                start=(k == 0),
                                stop=(k == KT - 1),
                            )
                        # DVE drains the main psum as soon as it is complete
                        # (adds the bias); frees the bank for the next tile
                        mb_sb = epool.tile([128, 512], F32, tag=f"mb{oh}")
                        nc.vector.tensor_tensor(
                            mb_sb[:, :], mps[:, :], bias_bc[:, osl], add
                        )
                        mset[oh] = mb_sb
                    lps = {}
                    for oh in range(NH):
                        osl = slice(oh * 512, (oh + 1) * 512)
                        lp_t = pp.tile([128, 512], F32, tag=f"lora{oh}")
                        lps[oh] = lp_t
                        mm = nc.tensor.matmul(
                            lp_t[:, :],
                            down_pers[:, tsl],
                            wu_sb[:, osl],
                            start=True,
                            stop=True,
                        )
                        if g == 1 and t == 0:
                            first_real.append(mm)
                    gset = {}
                    for oh in range(NH):
                        osl = slice(oh * 512, (oh + 1) * 512)
                        gps = pp.tile([128, 512], F32, tag=f"gate{oh}")
                        for j in range(KJ):
                            mm = nc.tensor.matmul(
                                gps[:, :],
                                x8_sb[:, j, :, tsl],
                                wg_h[oh][:, j, :, :],
                                start=(j == 0),
                                stop=(j == KJ - 1),
                                perf_mode=DR,
                            )
                            if g == 1 and t == 0 and j == 0:
                                first_real.append(mm)
                        gset[oh] = gps
                    last_tile = False
                    for oh in range(NH):
                        osl = slice(oh * 512, (oh + 1) * 512)
                        # sigmoid frees the gate psum; DVE mult frees the
                        # lora psum; gpsimd does the all-SBUF final add
                        # (DVE on the last tile: it is 0.6us faster and on
                        # the critical path there)
                        g_sb = epool.tile([128, 512], F32, tag=f"sig{oh}")
                        nc.scalar.activation(g_sb[:, :], gset[oh][:, :], sig,
                                             scale=1.0 / WS)
                        gl_sb = epool.tile([128, 512], F32, tag=f"gl{oh}")
                        nc.vector.tensor_tensor(
                            gl_sb[:, :], g_sb[:, :], lps[oh][:, :], mult
                        )
                        eng = nc.vector if last_tile else nc.gpsimd
                        eng.tensor_tensor(
                            out_sb[:, osl], gl_sb[:, :], mset[oh][:, :], add
                        )
                    nc.sync.dma_start(
                        out=out[tg0 + t * 128 : tg0 + (t + 1) * 128, :],
                        in_=out_sb[:, :],
                    )

            # --- phase 2: group 0's fp8 paths (downs, loras, gates) and the
            # final combine against the staged main+bias results ---
            dps = pp.tile([128, TG], F32, tag="dps", name="dps_g0")
            for j in range(KJ):
                nc.tensor.matmul(
                    dps[:, :],
                    wd_sb[:, j, :, :],
                    x0_8[:, j, :, :],
                    start=(j == 0),
                    stop=(j == KJ - 1),
                    perf_mode=DR,
                )
            nc.vector.tensor_copy(down_pers[0:R, :], dps[0:R, :])
            for t in range(TG // 128):
                tsl = slice(t * 128, (t + 1) * 128)
                out_sb = opool.tile([128, O], F32, tag="out", name=f"out_g0{t}")
                lps = {}
                for oh in range(NH):
                    osl = slice(oh * 512, (oh + 1) * 512)
                    lp_t = pp.tile([128, 512], F32, tag=f"lora{oh}",
                                   name=f"lp_g0{t}_{oh}")
                    lps[oh] = lp_t
                    nc.tensor.matmul(
                        lp_t[:, :],
                        down_pers[:, tsl],
                        wu_sb[:, osl],
                        start=True,
                        stop=True,
                    )
                gset = {}
                for oh in range(NH):
                    gps = pp.tile([128, 512], F32, tag=f"gate{oh}",
                                  name=f"gp_g0{t}_{oh}")
                    for j in range(KJ):
                        nc.tensor.matmul(
                            gps[:, :],
                            x0_8[:, j, :, tsl],
                            wg_h[oh][:, j, :, :],
                            start=(j == 0),
                            stop=(j == KJ - 1),
                            perf_mode=DR,
                        )
                    gset[oh] = gps
                last_tile = t == TG // 128 - 1
                for oh in range(NH):
                    osl = slice(oh * 512, (oh + 1) * 512)
                    g_sb = epool.tile([128, 512], F32, tag=f"sig{oh}",
                                      name=f"sg_g0{t}_{oh}")
                    nc.scalar.activation(g_sb[:, :], gset[oh][:, :], sig,
                                         scale=1.0 / WS)
                    gl_sb = epool.tile([128, 512], F32, tag=f"gl{oh}",
                                       name=f"gg_g0{t}_{oh}")
                    nc.vector.tensor_tensor(
                        gl_sb[:, :], g_sb[:, :], lps[oh][:, :], mult
                    )
                    eng = nc.vector if last_tile else nc.gpsimd
                    eng.tensor_tensor(
                        out_sb[:, osl], gl_sb[:, :], g0mb[(t, oh)][:, :], add
                    )
                nc.sync.dma_start(
                    out=out[t * 128 : (t + 1) * 128, :],
                    in_=out_sb[:, :],
                )

            # ordering-only deps: all junk matmuls precede the first matmul of
            # each psum chain so the PE queue never stalls behind real matmuls
            # waiting on input DMAs
            for fr in first_real:
                add_dep_helper(fr.ins, spin.ins, False,
                               "warmup before real matmuls")
    nc.compile()
    return nc


_NC_CACHE = None


def _get_nc():
    global _NC_CACHE
    if _NC_CACHE is None:
        _NC_CACHE = _build_nc()
    return _NC_CACHE


def _prep_inputs(x, W, b, W_down, W_up, W_gate, W_res):
    x = np.asarray(x, dtype=np.float32).reshape(TOK, I)
    # weights: [I, O] -> [half, 128p, kt(/pair), 512], k = kt*128 + p
    wcT = (np.asarray(W) + np.asarray(W_res)).T.astype(_BF16)
    wct = np.ascontiguousarray(
        wcT.reshape(KT, 128, NH, 512).transpose(2, 1, 0, 3)
    )
    wgT = (WS * np.asarray(W_gate)).T.astype(_F8E4)
    wg8 = np.ascontiguousarray(
        wgT.reshape(KJ, 2, 128, NH, 512).transpose(3, 2, 0, 1, 4)
    )
    # wd zero-padded from R=16 to 128 output columns
    wdT = np.zeros((I, 128), dtype=_F8E4)
    wdT[:, 0:R] = (WS * np.asarray(W_down)).T.astype(_F8E4)
    wd8 = np.ascontiguousarray(
        wdT.reshape(KJ, 2, 128, 128).transpose(2, 0, 1, 3)
    )
    # lora-up weights: scaling/WS folded in (zero-padded to K=128 on device)
    wut2 = np.ascontiguousarray(
        (SCALING / WS * np.asarray(W_up)).T.astype(_BF16)
    )  # [R, O]
    biasr = np.ascontiguousarray(np.asarray(b, dtype=np.float32).reshape(1, O))
    in_maps = []
    for c in range(N_CORES):
        xt_c = x[c * T : (c + 1) * T, :].T  # [I, T]
        # x: [I, T] -> [g, 128p, kt(/pair), TG], token t = g*TG + tau
        xtb_c = np.ascontiguousarray(
            xt_c.astype(_BF16).reshape(KT, 128, NG, TG).transpose(2, 1, 0, 3)
        )
        x8_c = np.ascontiguousarray(
            xt_c.astype(_F8E4).reshape(KJ, 2, 128, NG, TG)
            .transpose(3, 2, 0, 1, 4)
        )
        in_maps.append(
            {
                "xtb": xtb_c,
                "x8": x8_c,
                "wct": wct,
                "wg8": wg8,
                "wd8": wd8,
                "wut2": wut2,
                "biasr": biasr,
            }
        )
    return in_maps


def run(inputs, trace=False, **kwargs):
    """Build + run on the 8 NeuronCores. Returns (full_output, BassKernelResults)."""
    nc = _get_nc()
    in_maps = _prep_inputs(**inputs)
    res = run_bass_kernel_spmd(
        nc, in_maps, list(range(N_CORES)), trace=trace, **kwargs
    )
    shards = [res.results[c]["out"] for c in range(N_CORES)]
    full = np.concatenate(shards, axis=0).reshape(B, S, O)
    return full, res


def kernel(**inputs):
    out, _ = run(inputs, trace=False)
    return out


# revision 50
# speedup vs baseline: 1.0425x; 1.0425x over previous
"""EnhancedLoRALinear Trainium2 kernel (v2: bf16 main + fp8 DoubleRow gate/down).

Computes, for x:[4,8192,1024] and torch-style weights (out,in):
    out = x @ (W + W_res)^T + b + sigmoid(x @ W_gate^T) * (2 * (x @ W_down^T) @ W_up^T)

Strategy:
  - Data-parallel: the 32768 tokens are split across 8 NeuronCores (4096 each);
    the small weight matrices are replicated.
  - Algebraic fold: main + residual share one matmul with Wc = W + W_res.
  - Precision split (rel-err budget is 2e-2; measured 3.8e-3 on host):
      * main path in bf16 (x bf16 stationary, Wc bf16 moving) - full PE rate
        with FWL weight loads.
      * gate path in fp8 e4m3 with perf_mode=DoubleRow: K=1024 contraction in
        4 matmuls of K=256 (2 k-elements per cell). W_gate is scaled x64 so
        its entries leave the fp8 subnormal range; the sigmoid applies
        scale=1/64 to undo it. Sigmoid squashes the residual quantization
        error and the gate only multiplies the small LoRA term.
      * down-projection in fp8 DoubleRow too (W_down scaled x64; the 1/64
        plus the LoRA scaling 2.0 are folded into W_up host-side).
      * lora-up in bf16, with the two 512-wide output halves packed into
        concurrent row-tiled matmuls (K=16 each, rows 0-15 and 32-47).
  - DMA: weights issue on the Sync queue while x tiles issue on the Scalar
    queue (each dma_start costs ~1us of issue time on its engine; the v1
    kernel serialized everything on Sync which delayed the first real matmul
    to ~30us). Output tiles go back on Sync.
  - A short junk-matmul spin keeps the PE busy through the DMA prologue so
    the HAM clock gate is open when real work starts; ordering-only deps pin
    every junk matmul before the first matmul of each PSUM group.
"""

import ml_dtypes
import numpy as np

_BF16 = ml_dtypes.bfloat16
_F8E4 = ml_dtypes.float8_e4m3  # IEEE e4m3 (bias 7, max 240) == TRN FP8_EXP4

import concourse.bass as bass
import concourse.bacc as bacc
import concourse.mybir as mybir
import concourse.tile as tile
from concourse.bass_utils import run_bass_kernel_spmd
from concourse.tile_rust import add_dep_helper

N_CORES = 8
B, S = 4, 8192
TOK = B * S                  # 32768 tokens total
T = TOK // N_CORES           # 4096 tokens per core
I = 1024                     # in_features
O = 1024                     # out_features
R = 16                       # lora rank
SCALING = 2.0                # lora_alpha / r
KT = I // 128                # 8 bf16 contraction tiles
KJ = I // 256                # 4 fp8 DoubleRow contraction tiles
TG = 512                     # token group (down-projection batch)
NG = T // TG                 # 8 groups per core
NH = O // 512                # 2 output halves
WS = 64.0                    # fp8 weight scale (exact power of two)
N_JUNK = 12                  # HAM warm-up matmuls (cover the DMA prologue)

F32 = mybir.dt.float32
BF16 = mybir.dt.bfloat16
F8 = mybir.dt.float8e4
DR = mybir.MatmulPerfMode.DoubleRow


def _build_nc():
    nc = bacc.Bacc(None)

    # Pre-swizzled DRAM layouts, group-/half-major so every DMA moves 128
    # contiguous 4-8KB rows (small descriptors throttle the DMA queues):
    #   bf16 x:   [g, 128p, 8k, TG]     k = kt*128 + p
    #   fp8  x:   [g, 128p, 4j, 2i, TG] k = j*256 + i*128 + p
    #   wc/wg:    [half, 128p, ..., 512]
    xtb = nc.dram_tensor("xtb", [NG, 128, KT, TG], BF16, kind="ExternalInput")
    # group-0 bf16 x again, tile-major, so phase-1's first chunk gates on a
    # 0.25MB quarter instead of the whole 1MB group
    xtb0q = nc.dram_tensor("xtb0q", [TG // 128, 128, KT, 128], BF16,
                           kind="ExternalInput")
    x8 = nc.dram_tensor("x8", [NG, 128, KJ, 2, TG], F8, kind="ExternalInput")
    wct = nc.dram_tensor("wct", [NH, 128, KT, 512], BF16, kind="ExternalInput")
    wg8 = nc.dram_tensor("wg8", [NH, 128, KJ, 2, 512], F8,
                         kind="ExternalInput")
    wd8 = nc.dram_tensor("wd8", [128, KJ, 2, 128], F8, kind="ExternalInput")
    wut2 = nc.dram_tensor("wut2", [R, O], BF16, kind="ExternalInput")
    biasr = nc.dram_tensor("biasr", [1, O], F32, kind="ExternalInput")
    out = nc.dram_tensor("out", [T, O], F32, kind="ExternalOutput")

    sig = mybir.ActivationFunctionType.Sigmoid
    mult = mybir.AluOpType.mult
    add = mybir.AluOpType.add

    with tile.TileContext(nc) as tc:
        with (
            tc.tile_pool(name="wpool", bufs=1) as wpool,
            tc.tile_pool(name="xpool", bufs=3) as xpool,
            tc.tile_pool(name="opool", bufs=3) as opool,
            tc.tile_pool(name="epool", bufs=3) as epool,
            tc.tile_pool(name="psum", bufs=1, space="PSUM") as pp,
        ):
            # --- resident weights + group-0 x, split across BOTH DMA issue
            # queues (Sync + Scalar share ~400 GB/s; each alone gets ~200),
            # ordered by first use under the mains->loras->gates tile order.
            # wc/wg are separate per-half tiles so both DMA sides stay fully
            # contiguous (128 descriptors of 4-8KB each). ---
            wc_h = [wpool.tile([128, KT, 512], BF16, name=f"wc{h}")
                    for h in range(NH)]
            wg_h = [wpool.tile([128, KJ, 2, 512], F8, name=f"wg{h}")
                    for h in range(NH)]
            wd_sb = wpool.tile([128, KJ, 2, 128], F8)
            # lora operands are zero-padded to K=128 so the lora matmuls are
            # full-row (partial-row matmuls break LDWEIGHTS prefetching)
            wu_sb = wpool.tile([128, O], BF16)
            down_pers = wpool.tile([128, TG], BF16)
            bias_r = wpool.tile([1, O], F32)
            bias_bc = wpool.tile([128, O], F32)

            x0_8 = xpool.tile([128, KJ, 2, TG], F8, tag="x8", name="x8_0")
            x0_bq = [wpool.tile([128, KT, 128], BF16, name=f"xtb0q{t}")
                     for t in range(TG // 128)]
            nc.gpsimd.memset(wu_sb[:, :], 0.0)
            nc.gpsimd.memset(down_pers[:, :], 0.0)
            # sync queue: first wc half (feeds phase-1 immediately), then the
            # fp8 tensors needed from group 1 on
            nc.sync.dma_start(out=wc_h[0][:, :, :], in_=wct[0, :, :, :])
            nc.sync.dma_start(out=wg_h[0][:, :, :, :], in_=wg8[0, :, :, :, :])
            nc.sync.dma_start(out=wd_sb[:, :, :, :], in_=wd8[:, :, :, :])
            nc.sync.dma_start(out=x0_8[:, :, :, :], in_=x8[0, :, :, :, :])
            # scalar queue: bias first (the phase-1 drains need it), the four
            # bf16 x quarters for phase-1, second wc half, lora weights,
            # second wg half
            nc.scalar.dma_start(out=bias_r[:, :], in_=biasr[:, :])
            for t in range(TG // 128):
                nc.scalar.dma_start(out=x0_bq[t][:, :, :],
                                    in_=xtb0q[t, :, :, :])
            nc.scalar.dma_start(out=wc_h[1][:, :, :], in_=wct[1, :, :, :])
            nc.scalar.dma_start(out=wu_sb[0:R, :], in_=wut2[:, :])
            nc.scalar.dma_start(out=wg_h[1][:, :, :, :], in_=wg8[1, :, :, :, :])
            nc.gpsimd.partition_broadcast(bias_bc[:, :], bias_r[0:1, :])

            # HAM spin-up: junk matmuls keep the PE busy through the DMA
            # prologue so the clock gate is open before real compute starts
            junk = wpool.tile([128, 512], BF16)
            nc.gpsimd.memset(junk[:, :], 0.0)
            warm = pp.tile([128, 512], F32, tag="warm")
            spin = None
            for _ in range(N_JUNK):
                spin = nc.tensor.matmul(warm[:, :], junk[:, 0:128], junk[:, :],
                                        start=True, stop=True)
            first_real = []  # first matmul of each psum tag's first group

            # x-tile DMAs issue on the Scalar queue, one group ahead of use
            x_tiles = {}

            def issue_x(g):
                x8_t = xpool.tile([128, KJ, 2, TG], F8, tag="x8",
                                  name=f"x8_{g}")
                nc.scalar.dma_start(
                    out=x8_t[:, :, :, :], in_=x8[g, :, :, :, :]
                )
                xt_t = xpool.tile([128, KT, TG], BF16, tag="xtb",
                                  name=f"xtb{g}")
                nc.scalar.dma_start(
                    out=xt_t[:, :, :], in_=xtb[g, :, :, :]
                )
                x_tiles[g] = (xt_t, x8_t)

            # --- phase 1: group-0 mains only. Only wc and bf16-x are needed,
            # so the PE gets real work while the rest of the 5.5MB prologue
            # is still in flight (the prologue is HBM-bandwidth-bound).
            # main+bias results are staged in SBUF; group 0's fp8 paths and
            # the final combine run as phase 2 after group 7. ---
            # rotate over six psum banks (the gate/lora tags are free during
            # phase 1) so the 8-matmul bursts pipeline instead of
            # serializing on the DVE drain of a single bank
            P1_BANKS = ["main0", "main1", "gate0", "gate1", "lora0", "lora1"]
            g0mb = {}
            ci = 0
            for oh in range(NH):
                osl = slice(oh * 512, (oh + 1) * 512)
                for t in range(TG // 128):
                    mps = pp.tile([128, 512], F32, tag=P1_BANKS[ci % 6],
                                  name=f"p1ps{ci}")
                    for k in range(KT):
                        mm = nc.tensor.matmul(
                            mps[:, :],
                            x0_bq[t][:, k, :],
                            wc_h[oh][:, k, :],
                            start=(k == 0),
                            stop=(k == KT - 1),
                        )
                        if ci < 6 and k == 0:
                            first_real.append(mm)
                    st_sb = wpool.tile([128, 512], F32, name=f"g0mb{t}_{oh}")
                    nc.vector.tensor_tensor(
                        st_sb[:, :], mps[:, :], bias_bc[:, osl], add
                    )
                    g0mb[(t, oh)] = st_sb
                    ci += 1

            issue_x(1)
            issue_x(2)
            for g in range(1, NG):
                tg0 = g * TG
                xt_sb, x8_sb = x_tiles.pop(g)

                # LoRA down-projection (64x scaled) for the whole group.
                # wd is zero-padded to 128 output columns so its LDWEIGHTS
                # keeps the full column mask (no prefetch break).
                dps = pp.tile([128, TG], F32, tag="dps")
                for j in range(KJ):
                    mm = nc.tensor.matmul(
                        dps[:, :],
                        wd_sb[:, j, :, :],
                        x8_sb[:, j, :, :],
                        start=(j == 0),
                        stop=(j == KJ - 1),
                        perf_mode=DR,
                    )
                    if g == 1 and j == 0:
                        first_real.append(mm)
                nc.vector.tensor_copy(down_pers[0:R, :], dps[0:R, :])

                for t in range(TG // 128):
                    # prefetch the next-next group's x mid-group, after the
                    # DGE queue has drained (issuing early blocks the scalar
                    # engine on descriptor backpressure, delaying sigmoids)
                    if t == 2 and g + 2 < NG:
                        issue_x(g + 2)
                    tsl = slice(t * 128, (t + 1) * 128)
                    out_sb = opool.tile([128, O], F32, tag="out")
                    # tile order: all bf16 work first (mains, loras), then
                    # the fp8 gates contiguously -- the fp8-DR weight path
                    # entry costs ~190ns, so pay it once per tile, adjacent
                    # to the next group's fp8 down-projection
                    mset = {}
                    for oh in range(NH):
                        osl = slice(oh * 512, (oh + 1) * 512)
                        mps = pp.tile([128, 512], F32, tag=f"main{oh}")
                        for k in range(KT):
                            mm = nc.tensor.matmul(
                                mps[:, :],
                                xt_sb[:, k, tsl],
                                wc_h[oh][:, k, :],
                                start=(k == 0),
                                stop=(k == KT - 1),
                            )
                        # DVE drains the main psum as soon as it is complete
                        # (adds the bias); frees the bank for the next tile
                        mb_sb = epool.tile([128, 512], F32, tag=f"mb{oh}")
                        nc.vector.tensor_tensor(
                            mb_sb[:, :], mps[:, :], bias_bc[:, osl], add
                        )
                        mset[oh] = mb_sb
                    lps = {}
                    for oh in range(NH):
                        osl = slice(oh * 512, (oh + 1) * 512)
                        lp_t = pp.tile([128, 512], F32, tag=f"lora{oh}")
                        lps[oh] = lp_t
                        mm = nc.tensor.matmul(
                            lp_t[:, :],
                            down_pers[:, tsl],
                            wu_sb[:, osl],
                            start=True,
                            stop=True,
                        )
                        if g == 1 and t == 0:
                            first_real.append(mm)
                    gset = {}
                    for oh in range(NH):
                        osl = slice(oh * 512, (oh + 1) * 512)
                        gps = pp.tile([128, 512], F32, tag=f"gate{oh}")
                        for j in range(KJ):
                            mm = nc.tensor.matmul(
                                gps[:, :],
                                x8_sb[:, j, :, tsl],
                                wg_h[oh][:, j, :, :],
                                start=(j == 0),
                                stop=(j == KJ - 1),
                                perf_mode=DR,
                            )
                            if g == 1 and t == 0 and j == 0:
                                first_real.append(mm)
                        gset[oh] = gps
                    last_tile = False
                    for oh in range(NH):
                        osl = slice(oh * 512, (oh + 1) * 512)
                        # sigmoid frees the gate psum; DVE mult frees the
                        # lora psum; gpsimd does the all-SBUF final add
                        # (DVE on the last tile: it is 0.6us faster and on
                        # the critical path there)
                        g_sb = epool.tile([128, 512], F32, tag=f"sig{oh}")
                        nc.scalar.activation(g_sb[:, :], gset[oh][:, :], sig,
                                             scale=1.0 / WS)
                        gl_sb = epool.tile([128, 512], F32, tag=f"gl{oh}")
                        nc.vector.tensor_tensor(
                            gl_sb[:, :], g_sb[:, :], lps[oh][:, :], mult
                        )
                        eng = nc.vector if last_tile else nc.gpsimd
                        eng.tensor_tensor(
                            out_sb[:, osl], gl_sb[:, :], mset[oh][:, :], add
                        )
                    nc.sync.dma_start(
                        out=out[tg0 + t * 128 : tg0 + (t + 1) * 128, :],
                        in_=out_sb[:, :],
                    )

            # --- phase 2: group 0's fp8 paths (downs, loras, gates) and the
            # final combine against the staged main+bias results ---
            dps = pp.tile([128, TG], F32, tag="dps", name="dps_g0")
            for j in range(KJ):
                nc.tensor.matmul(
                    dps[:, :],
                    wd_sb[:, j, :, :],
                    x0_8[:, j, :, :],
                    start=(j == 0),
                    stop=(j == KJ - 1),
                    perf_mode=DR,
                )
            nc.vector.tensor_copy(down_pers[0:R, :], dps[0:R, :])
            for t in range(TG // 128):
                tsl = slice(t * 128, (t + 1) * 128)
                out_sb = opool.tile([128, O], F32, tag="out", name=f"out_g0{t}")
                lps = {}
                for oh in range(NH):
                    osl = slice(oh * 512, (oh + 1) * 512)
                    lp_t = pp.tile([128, 512], F32, tag=f"lora{oh}",
                                   name=f"lp_g0{t}_{oh}")
                    lps[oh] = lp_t
                    nc.tensor.matmul(
                        lp_t[:, :],
                        down_pers[:, tsl],
                        wu_sb[:, osl],
                        start=True,
                        stop=True,
                    )
                gset = {}
                for oh in range(NH):
                    gps = pp.tile([128, 512], F32, tag=f"gate{oh}",
                                  name=f"gp_g0{t}_{oh}")
                    for j in range(KJ):
                        nc.tensor.matmul(
                            gps[:, :],
                            x0_8[:, j, :, tsl],
                            wg_h[oh][:, j, :, :],
                            start=(j == 0),
                            stop=(j == KJ - 1),
                            perf_mode=DR,
                        )
                    gset[oh] = gps
                last_tile = t == TG // 128 - 1
                for oh in range(NH):
                    osl = slice(oh * 512, (oh + 1) * 512)
                    g_sb = epool.tile([128, 512], F32, tag=f"sig{oh}",
                                      name=f"sg_g0{t}_{oh}")
                    nc.scalar.activation(g_sb[:, :], gset[oh][:, :], sig,
                                         scale=1.0 / WS)
                    gl_sb = epool.tile([128, 512], F32, tag=f"gl{oh}",
                                       name=f"gg_g0{t}_{oh}")
                    nc.vector.tensor_tensor(
                        gl_sb[:, :], g_sb[:, :], lps[oh][:, :], mult
                    )
                    eng = nc.vector if last_tile else nc.gpsimd
                    eng.tensor_tensor(
                        out_sb[:, osl], gl_sb[:, :], g0mb[(t, oh)][:, :], add
                    )
                nc.sync.dma_start(
                    out=out[t * 128 : (t + 1) * 128, :],
                    in_=out_sb[:, :],
                )

            # ordering-only deps: all junk matmuls precede the first matmul of
            # each psum chain so the PE queue never stalls behind real matmuls
            # waiting on input DMAs
            for fr in first_real:
                add_dep_helper(fr.ins, spin.ins, False,
                               "warmup before real matmuls")
    nc.compile()
    return nc


_NC_CACHE = None


def _get_nc():
    global _NC_CACHE
    if _NC_CACHE is None:
        _NC_CACHE = _build_nc()
    return _NC_CACHE


def _prep_inputs(x, W, b, W_down, W_up, W_gate, W_res):
    x = np.asarray(x, dtype=np.float32).reshape(TOK, I)
    # weights: [I, O] -> [half, 128p, kt(/pair), 512], k = kt*128 + p
    wcT = (np.asarray(W) + np.asarray(W_res)).T.astype(_BF16)
    wct = np.ascontiguousarray(
        wcT.reshape(KT, 128, NH, 512).transpose(2, 1, 0, 3)
    )
    wgT = (WS * np.asarray(W_gate)).T.astype(_F8E4)
    wg8 = np.ascontiguousarray(
        wgT.reshape(KJ, 2, 128, NH, 512).transpose(3, 2, 0, 1, 4)
    )
    # wd zero-padded from R=16 to 128 output columns
    wdT = np.zeros((I, 128), dtype=_F8E4)
    wdT[:, 0:R] = (WS * np.asarray(W_down)).T.astype(_F8E4)
    wd8 = np.ascontiguousarray(
        wdT.reshape(KJ, 2, 128, 128).transpose(2, 0, 1, 3)
    )
    # lora-up weights: scaling/WS folded in (zero-padded to K=128 on device)
    wut2 = np.ascontiguousarray(
        (SCALING / WS * np.asarray(W_up)).T.astype(_BF16)
    )  # [R, O]
    biasr = np.ascontiguousarray(np.asarray(b, dtype=np.float32).reshape(1, O))
    in_maps = []
    for c in range(N_CORES):
        xt_c = x[c * T : (c + 1) * T, :].T  # [I, T]
        # x: [I, T] -> [g, 128p, kt(/pair), TG], token t = g*TG + tau
        xtb_c = np.ascontiguousarray(
            xt_c.astype(_BF16).reshape(KT, 128, NG, TG).transpose(2, 1, 0, 3)
        )
        # tile-major copy of group 0 for the phase-1 quarter loads
        xtb0q_c = np.ascontiguousarray(
            xtb_c[0].reshape(128, KT, TG // 128, 128).transpose(2, 0, 1, 3)
        )
        x8_c = np.ascontiguousarray(
            xt_c.astype(_F8E4).reshape(KJ, 2, 128, NG, TG)
            .transpose(3, 2, 0, 1, 4)
        )
        in_maps.append(
            {
                "xtb": xtb_c,
                "xtb0q": xtb0q_c,
                "x8": x8_c,
                "wct": wct,
                "wg8": wg8,
                "wd8": wd8,
                "wut2": wut2,
                "biasr": biasr,
            }
        )
    return in_maps


def run(inputs, trace=False, **kwargs):
    """Build + run on the 8 NeuronCores. Returns (full_output, BassKernelResults)."""
    nc = _get_nc()
    in_maps = _prep_inputs(**inputs)
    res = run_bass_kernel_spmd(
        nc, in_maps, list(range(N_CORES)), trace=trace, **kwargs
    )
    shards = [res.results[c]["out"] for c in range(N_CORES)]
    full = np.concatenate(shards, axis=0).reshape(B, S, O)
    return full, res


def kernel(**inputs):
    out, _ = run(inputs, trace=False)
    return out
